# revision 8
# baseline (speedup 1.0000x reference)
"""PinSAGE 2-layer GNN on 8 Trainium2 NeuronCores.

Shapes (fixed): N0=400000, N1=100000, N2=20000, F=16, D=128, 8 cores.

Launch A (source-sharded): t = relu(emb @ Wa1 + ba1) over 50000-row shards,
    feature-major matmuls (host supplies emb^T), PE-transpose back to
    row-major so later launches can gather 512B rows.

Launch B (destination-sharded, 12500 dest/core padded to 12800):
    Neighbor aggregation = indirect row gather + weighted sum. The HW gather
    primitive (InstDMAGatherAnt) takes int16 indices, so draws are bucketed
    into 32768-row source windows. Draws are sorted by (256-dest-block,
    window); each (block, window) cell is padded to a multiple of 128 slots
    with dummy row-0/weight-0 draws, sized exactly from the actual inputs
    (max across cores) so one SPMD program fits all 8 cores. Gathers run on
    all 4 SWDGE queues (~4x descriptor throughput). Aggregation runs on the
    PE: per 128-slot tile, stationary = gathered rows, moving = a [128, 256]
    routing matrix built on-device by one DVE tensor_scalar(is_equal, mult)
    from host-sent per-slot (dest-col, weight) pairs; agg^T accumulates
    feature-major in PSUM.
    Self rows use a two-hop gather: window-sorted gather into a <32768-row
    HBM staging table, then a dest-ordered int16 re-gather from it.
    Encoder h1^T = relu(We_s^T self^T + We_a^T agg^T + be), tail
    t2^T = relu(Wa2^T h1^T + ba2), both PE-transposed to row-major.

Launch C: same pipeline for layer 2 (src tables = t2/h1, 2500 dest/core).

Host work is index arithmetic / layout only; all feature data moves and is
computed on device.
"""

import os
import numpy as np

from concourse import bass, bacc, mybir, tile
from concourse.bass_utils import run_bass_kernel_spmd
from concourse.masks import make_identity
import concourse.tile_sem_assignment as _tsa

# Tile assigns the 8 DMASW completion-sem lanes round-robin over Pool DMA
# instructions, but a lane must stay locked to one SWDGE queue. With
# multi-queue gathers, derive the lane from queue_num (2 lanes per queue).
if not getattr(_tsa, "_queue_aware_lanes", False):
    _orig_assign_tick = _tsa.TileClockTick._assign_tick

    def _assign_tick_queue_aware(self, inst):
        qn = getattr(inst, "queue_num", None)
        if (qn is not None and inst.engine == mybir.EngineType.Pool
                and isinstance(inst, _tsa.DMAInst)):
            tog = getattr(self, "_qlane_toggle", None)
            if tog is None:
                tog = self._qlane_toggle = {}
            t = tog.get(qn, 0)
            tog[qn] = t ^ 1
            self.next_sw_dma_idx = 2 * qn + t
        return _orig_assign_tick(self, inst)

    _tsa.TileClockTick._assign_tick = _assign_tick_queue_aware
    _tsa._queue_aware_lanes = True

F32 = mybir.dt.float32
I16 = mybir.dt.int16

N0, N1, N2, F, D = 400000, 100000, 20000, 16, 128
NCORES = 8
WSZ = 32768          # int16-addressable source window
DBLK = 256           # dest block = routing matmul N
MAXCALL = 1024       # max indices per dma_gather call (descriptor ring)

PA = 50176           # launch A padded shard rows (392*128)
NA = N0 // NCORES
DB_DEST = 12800      # launch B padded dest/core
DC_DEST = 2560       # launch C padded dest/core

LAST_EXEC_TIMES = {}
LAST_PROFILES = {}


def cdiv(a, b):
    return -(-a // b)


def _run(nc, in_maps, label):
    want_trace = bool(os.environ.get("BASS_TRACE"))
    res = run_bass_kernel_spmd(nc, in_maps, core_ids=list(range(NCORES)),
                               trace=want_trace)
    if want_trace:
        LAST_EXEC_TIMES[label] = res.exec_time_ns
        if res.instructions_and_trace is not None:
            LAST_PROFILES[label] = (res.instructions_and_trace[1],
                                    res.profile_json)
    return res.results


def idx16_cols(idxs):
    """Flat int16 index list (len % 16 == 0) -> [128, len/16] wrapped layout:
    index i read from [i % 16, i // 16]; replicated to all 8 Q7 groups."""
    a = np.asarray(idxs, dtype=np.int16)
    t16 = a.reshape(-1, 16).T  # [16, cols]
    return np.tile(t16, (8, 1))  # [128, cols]


# ===================================================================
# Launch A
# ===================================================================
def _build_launch_a(pa=PA):
    nc = bacc.Bacc("TRN2", target_bir_lowering=False, debug=False)
    embT = nc.dram_tensor("embT", [D, pa], F32, kind="ExternalInput").ap()
    Wa1 = nc.dram_tensor("Wa1", [D, D], F32, kind="ExternalInput").ap()
    ba1 = nc.dram_tensor("ba1", [D, 1], F32, kind="ExternalInput").ap()
    t_out = nc.dram_tensor("t", [pa, D], F32, kind="ExternalOutput").ap()

    CH = 8            # 8 node-tiles (1024 nodes) per DMA chunk
    nchunks = pa // (D * CH)

    with tile.TileContext(nc) as tc:
        with tc.tile_pool(name="const", bufs=1) as cpool, \
             tc.tile_pool(name="sb", bufs=3) as pool, \
             tc.tile_pool(name="ps", bufs=4, space="PSUM") as psum:
            ident = cpool.tile([D, D], F32)
            make_identity(nc, ident[:])
            wa = cpool.tile([D, D], F32)
            nc.sync.dma_start(out=wa[:], in_=Wa1[:])
            ba = cpool.tile([D, 1], F32)
            nc.sync.dma_start(out=ba[:], in_=ba1[:])

            for i in range(nchunks):
                x = pool.tile([D, CH * D], F32, tag="x")
                nc.sync.dma_start(
                    out=x[:], in_=embT[:, i * CH * D:(i + 1) * CH * D])
                tn = pool.tile([D, CH, D], F32, tag="tn")
                for j in range(0, CH, 4):
                    ps1 = psum.tile([D, 4 * D], F32, tag="mm")
                    nc.tensor.matmul(out=ps1[:],
                                     lhsT=wa[:],
                                     rhs=x[:, j * D:(j + 4) * D],
                                     start=True, stop=True)
                    tT = pool.tile([D, 4 * D], F32, tag="tT")
                    nc.scalar.activation(
                        out=tT[:], in_=ps1[:],
                        func=mybir.ActivationFunctionType.Relu, bias=ba[:])
                    for k in range(4):
                        ps2 = psum.tile([D, D], F32, tag="tr")
                        nc.tensor.transpose(out=ps2[:],
                                            in_=tT[:, k * D:(k + 1) * D],
                                            identity=ident[:])
                        nc.vector.tensor_copy(out=tn[:, j + k, :], in_=ps2[:])
                nc.sync.dma_start(
                    out=t_out[i * CH * D:(i + 1) * CH * D, :].rearrange(
                        "(j p) f -> p j f", p=D),
                    in_=tn[:])
    nc.compile()
    return nc


# ===================================================================
# Plan for launches B/C: window-bucketed gather layout
# ===================================================================
class Plan:
    pass


def _build_plan(nidx_all, nw_all, sidx_all, ndest, nsrc, nself):
    """nidx_all [C, ndest, F] int64, nw_all [C, ndest, F] f32 (normalized,
    pads zero), sidx_all [C, ndest] int64. Returns a Plan with a static
    structure (shared by all cores, sized from per-cell maxima) plus
    per-core index/weight tensors."""
    C = NCORES
    W = cdiv(nsrc, WSZ)
    nblk = ndest // DBLK
    npair = cdiv(nblk, 2)
    draws = ndest * F

    dest = np.repeat(np.arange(ndest), F)
    blk_of_draw = dest >> 8

    # ---- per-core cell sort (cell = (block, window)) ----
    cell_sorted = []   # per core: (order, cellid)
    counts = np.zeros((C, nblk, W), dtype=np.int64)
    for c in range(C):
        src = nidx_all[c].reshape(-1)
        w = src >> 15
        cellid = blk_of_draw * W + w
        order = np.argsort(cellid, kind="stable")  # keeps (dest, f) order
        cell_sorted.append((order, cellid))
        cnt = np.bincount(cellid, minlength=nblk * W).reshape(nblk, W)
        counts[c] = cnt

    S = (cdiv(counts.max(axis=0), 128) * 128).astype(np.int64)  # [nblk, W]

    # ---- static layout: per pair, per window: cells of its (1-2) blocks ----
    plan = Plan()
    plan.W, plan.nblk, plan.npair, plan.ndest, plan.nsrc = W, nblk, npair, ndest, nsrc
    plan.pairs = []
    cw_tiles = []      # traversal order of (pair, w, b, tile) -> cw column
    idx_col = 0        # column offset into the global idx16 tensor
    for p in range(npair):
        blocks = [b for b in (2 * p, 2 * p + 1) if b < nblk]
        info = {"blocks": {b: [] for b in blocks}, "calls": [],
                "idx_col0": idx_col, "nslots": 0}
        slot_off = 0
        for w in range(W):
            ncall = int(sum(S[b, w] for b in blocks))
            if ncall == 0:
                continue
            # split at MAXCALL boundaries (cell sizes are 128-multiples)
            seg = []
            acc = 0
            for b in blocks:
                sbw = int(S[b, w])
                if sbw:
                    info["blocks"][b].append((w, slot_off + acc, sbw))
                    acc += sbw
            start = 0
            while start < ncall:
                n = min(MAXCALL, ncall - start)
                info["calls"].append(
                    dict(w=w, n=n, dst_slot=slot_off + start,
                         idx_col=idx_col + start // 16))
                start += n
            slot_off += ncall
            idx_col += ncall // 16
        info["nslots"] = slot_off
        # tile list per block (slot offsets of each 128-slot tile)
        for b in blocks:
            tl = []
            for (w, off, sbw) in info["blocks"][b]:
                for t0 in range(off, off + sbw, 128):
                    tl.append((t0, len(cw_tiles)))
                    cw_tiles.append(None)  # placeholder; filled per core
            info["blocks"][b] = tl
        plan.pairs.append(info)
    plan.idx_cols_total = idx_col
    plan.ntiles_total = len(cw_tiles)
    plan.maxpair_slots = max(i["nslots"] for i in plan.pairs)
    plan.maxpair_idxcols = max(
        (sum(cc["n"] for cc in i["calls"])) // 16 for i in plan.pairs)
    plan.maxpair_tiles = max(
        sum(len(tl) for tl in i["blocks"].values()) for i in plan.pairs)

    # ---- per-core idx16 + cw tensors ----
    plan.idx16 = np.zeros((C, 128, idx_col), dtype=np.int16)
    plan.cw = np.zeros((C, 128, 2 * plan.ntiles_total), dtype=np.float32)
    for c in range(C):
        order, cellid = cell_sorted[c]
        src = nidx_all[c].reshape(-1)
        wt = nw_all[c].reshape(-1)
        dst_col = dest & (DBLK - 1)
        # per cell: sorted draw lists
        cell_start = np.zeros(nblk * W + 1, dtype=np.int64)
        np.cumsum(np.bincount(cellid, minlength=nblk * W), out=cell_start[1:])
        for p, info in enumerate(plan.pairs):
            # idx stream: per window, blocks in order, cell draws + pads
            stream = []
            for w in range(W):
                for b in (2 * p, 2 * p + 1):
                    if b >= nblk or S[b, w] == 0:
                        continue
                    cid = b * W + w
                    dr = order[cell_start[cid]:cell_start[cid + 1]]
                    loc = src[dr] - (w << 15)
                    pad = int(S[b, w]) - len(dr)
                    stream.append(loc)
                    if pad:
                        stream.append(np.zeros(pad, dtype=np.int64))
            if stream:
                flat = np.concatenate(stream)
                assert len(flat) == info["nslots"]
                cols = idx16_cols(flat)
                c0 = info["idx_col0"]
                plan.idx16[c, :, c0:c0 + len(flat) // 16] = cols
        # fill cw via a slot->draw map per pair
        for p, info in enumerate(plan.pairs):
            nslots = info["nslots"]
            slot_dstcol = np.zeros(nslots, dtype=np.float32)
            slot_w = np.zeros(nslots, dtype=np.float32)
            pos = 0
            for w in range(W):
                for b in (2 * p, 2 * p + 1):
                    if b >= nblk or S[b, w] == 0:
                        continue
                    cid = b * W + w
                    dr = order[cell_start[cid]:cell_start[cid + 1]]
                    n = len(dr)
                    slot_dstcol[pos:pos + n] = dst_col[dr]
                    slot_w[pos:pos + n] = wt[dr]
                    pos += int(S[b, w])
            for b, tl in info["blocks"].items():
                for (t0, cwcol) in tl:
                    plan.cw[c, :, 2 * cwcol] = slot_dstcol[t0:t0 + 128]
                    plan.cw[c, :, 2 * cwcol + 1] = slot_w[t0:t0 + 128]

    # ---- self rows: two-hop ----
    WS = cdiv(nself, WSZ)
    plan.WS = WS
    scnt = np.zeros((C, WS), dtype=np.int64)
    sorders = []
    for c in range(C):
        sw = sidx_all[c] >> 15
        so = np.argsort(sw, kind="stable")
        sorders.append(so)
        scnt[c] = np.bincount(sw, minlength=WS)
    SW = (cdiv(scnt.max(axis=0), 128) * 128).astype(np.int64)  # [WS]
    plan.SW = SW
    plan.stage_rows = int(SW.sum())
    soff = np.concatenate([[0], np.cumsum(SW)])
    plan.self_calls = []
    for w in range(WS):
        if SW[w] == 0:
            continue
        start = 0
        while start < SW[w]:
            n = int(min(MAXCALL, SW[w] - start))
            plan.self_calls.append(
                dict(w=w, n=n, dst_row=int(soff[w] + start),
                     idx_col=int((soff[w] + start) // 16)))
            start += n
    plan.sidx16 = np.zeros((C, 128, plan.stage_rows // 16), dtype=np.int16)
    plan.sre16 = np.zeros((C, 128, ndest // 16), dtype=np.int16)
    for c in range(C):
        so = sorders[c]
        sw = (sidx_all[c] >> 15)[so]
        vals = sidx_all[c][so] - (sw << 15)
        stream = []
        p_stage = np.zeros(ndest, dtype=np.int64)
        for w in range(WS):
            n = int(scnt[c, w])
            seg = vals[np.searchsorted(sw, w):np.searchsorted(sw, w + 1)]
            dests = so[np.searchsorted(sw, w):np.searchsorted(sw, w + 1)]
            p_stage[dests] = soff[w] + np.arange(n)
            stream.append(seg)
            stream.append(np.zeros(int(SW[w]) - n, dtype=np.int64))
        flat = np.concatenate(stream) if stream else np.zeros(0)
        plan.sidx16[c] = idx16_cols(flat)
        plan.sre16[c] = idx16_cols(p_stage)
    return plan


# ===================================================================
# Launch B/C kernel builder
# ===================================================================
def _build_launch_bc(plan, nsrc, nself, with_tail):
    ndest, W, nblk = plan.ndest, plan.W, plan.nblk

    nc = bacc.Bacc("TRN2", target_bir_lowering=False, debug=False,
                   num_swdge_queues=4)
    tsrc = nc.dram_tensor("tsrc", [nsrc, D], F32, kind="ExternalInput").ap()
    sself = nc.dram_tensor("sself", [nself, D], F32, kind="ExternalInput").ap()
    gidx = nc.dram_tensor("gidx", [128, plan.idx_cols_total], I16,
                          kind="ExternalInput").ap()
    cw = nc.dram_tensor("cw", [128, 2 * plan.ntiles_total], F32,
                        kind="ExternalInput").ap()
    sidx = nc.dram_tensor("sidx", [128, plan.stage_rows // 16], I16,
                          kind="ExternalInput").ap()
    sre = nc.dram_tensor("sre", [128, ndest // 16], I16,
                         kind="ExternalInput").ap()
    iota = nc.dram_tensor("iota", [128, DBLK], F32, kind="ExternalInput").ap()
    wes = nc.dram_tensor("wes", [D, D], F32, kind="ExternalInput").ap()
    wea = nc.dram_tensor("wea", [D, D], F32, kind="ExternalInput").ap()
    be = nc.dram_tensor("be", [D, 1], F32, kind="ExternalInput").ap()
    h_out = nc.dram_tensor("h", [ndest, D], F32, kind="ExternalOutput").ap()
    if with_tail:
        wa2 = nc.dram_tensor("wa2", [D, D], F32, kind="ExternalInput").ap()
        ba2 = nc.dram_tensor("ba2", [D, 1], F32, kind="ExternalInput").ap()
        t2_out = nc.dram_tensor("t2", [ndest, D], F32,
                                kind="ExternalOutput").ap()
    stage = nc.dram_tensor("stage", [plan.stage_rows, D], F32).ap()

    qi = [0]

    def next_q():
        q = qi[0] % 4
        qi[0] += 1
        return q

    def win_view(tensor, rows, w):
        lo = w * WSZ
        hi = min(rows, lo + WSZ)
        return tensor[lo:hi, :]

    relu = mybir.ActivationFunctionType.Relu

    with tile.TileContext(nc) as tc:
        with tc.tile_pool(name="const", bufs=1) as cpool, \
             tc.tile_pool(name="gt", bufs=2) as gpool, \
             tc.tile_pool(name="sb", bufs=2) as pool, \
             tc.tile_pool(name="rt", bufs=4) as rpool, \
             tc.tile_pool(name="psA", bufs=2, space="PSUM") as psA, \
             tc.tile_pool(name="psB", bufs=2, space="PSUM") as psB, \
             tc.tile_pool(name="psC", bufs=2, space="PSUM") as psC, \
             tc.tile_pool(name="psT", bufs=2, space="PSUM") as psT:
            ident = cpool.tile([D, D], F32)
            make_identity(nc, ident[:])
            w_self = cpool.tile([D, D], F32)
            nc.sync.dma_start(out=w_self[:], in_=wes[:])
            w_agg = cpool.tile([D, D], F32)
            nc.sync.dma_start(out=w_agg[:], in_=wea[:])
            be_sb = cpool.tile([D, 1], F32)
            nc.sync.dma_start(out=be_sb[:], in_=be[:])
            iota_sb = cpool.tile([128, DBLK], F32)
            nc.sync.dma_start(out=iota_sb[:], in_=iota[:])
            if with_tail:
                wa2_sb = cpool.tile([D, D], F32)
                nc.sync.dma_start(out=wa2_sb[:], in_=wa2[:])
                ba2_sb = cpool.tile([D, 1], F32)
                nc.sync.dma_start(out=ba2_sb[:], in_=ba2[:])
            sre_sb = cpool.tile([128, ndest // 16], I16)
            nc.sync.dma_start(out=sre_sb[:], in_=sre[:])

            # ---- phase S1: self rows -> window-sorted staging table ----
            sidx_sb = cpool.tile([128, plan.stage_rows // 16], I16)
            nc.sync.dma_start(out=sidx_sb[:], in_=sidx[:])
            for call in plan.self_calls:
                n = call["n"]
                st = pool.tile([128, MAXCALL // 128, D], F32, tag="s1")
                nc.gpsimd.dma_gather(
                    out_ap=st[:, :n // 128, :],
                    in_ap=win_view(sself, nself, call["w"]),
                    idxs_ap=sidx_sb[:, call["idx_col"]:call["idx_col"] + n // 16],
                    num_idxs=n, num_idxs_reg=n, elem_size=D,
                    queue_num=next_q())
                nc.sync.dma_start(
                    out=stage[call["dst_row"]:call["dst_row"] + n, :].rearrange(
                        "(j p) f -> p j f", p=128),
                    in_=st[:, :n // 128, :])

            # ---- main loop over block pairs ----
            for p, info in enumerate(plan.pairs):
                nslots = info["nslots"]
                if nslots == 0:
                    continue
                icols = nslots // 16
                gi = pool.tile([128, plan.maxpair_idxcols], I16, tag="gi")
                c0 = info["idx_col0"]
                nc.sync.dma_start(out=gi[:, :icols],
                                  in_=gidx[:, c0:c0 + icols])
                gt = gpool.tile([128, plan.maxpair_slots // 128, D], F32,
                                tag="gt")
                for call in info["calls"]:
                    n, ds = call["n"], call["dst_slot"]
                    nc.gpsimd.dma_gather(
                        out_ap=gt[:, ds // 128:(ds + n) // 128, :],
                        in_ap=win_view(tsrc, nsrc, call["w"]),
                        idxs_ap=gi[:, call["idx_col"] - c0:
                                   call["idx_col"] - c0 + n // 16],
                        num_idxs=n, num_idxs_reg=n, elem_size=D,
                        queue_num=next_q())
                # cw chunk for this pair
                tiles_all = [t for b in info["blocks"]
                             for t in info["blocks"][b]]
                cw0 = min(t[1] for t in tiles_all)
                cwn = len(tiles_all)
                cwt = pool.tile([128, 2 * plan.maxpair_tiles], F32, tag="cw")
                nc.sync.dma_start(out=cwt[:, :2 * cwn],
                                  in_=cw[:, 2 * cw0:2 * (cw0 + cwn)])

                for b, tl in info["blocks"].items():
                    if not tl:
                        continue
                    # aggregation
                    agg_ps = psA.tile([128, DBLK], F32, tag="agg")
                    for j, (t0, cwcol) in enumerate(tl):
                        route = rpool.tile([128, DBLK], F32, tag="route")
                        lc = 2 * (cwcol - cw0)
                        nc.vector.tensor_scalar(
                            out=route[:], in0=iota_sb[:],
                            scalar1=cwt[:, lc:lc + 1],
                            scalar2=cwt[:, lc + 1:lc + 2],
                            op0=mybir.AluOpType.is_equal,
                            op1=mybir.AluOpType.mult)
                        nc.tensor.matmul(
                            out=agg_ps[:], lhsT=gt[:, t0 // 128, :],
                            rhs=route[:], start=(j == 0),
                            stop=(j == len(tl) - 1))
                    aggT = pool.tile([128, DBLK], F32, tag="aggT")
                    nc.vector.tensor_copy(out=aggT[:], in_=agg_ps[:])

                    # self re-gather (dest order) + transpose
                    sg = pool.tile([128, DBLK // 128, D], F32, tag="sg")
                    nc.gpsimd.dma_gather(
                        out_ap=sg[:], in_ap=stage[:],
                        idxs_ap=sre_sb[:, b * DBLK // 16:(b + 1) * DBLK // 16],
                        num_idxs=DBLK, num_idxs_reg=DBLK, elem_size=D,
                        queue_num=next_q())
                    sps = psB.tile([128, DBLK], F32, tag="selfps")
                    for k in range(DBLK // 128):
                        nc.tensor.transpose(out=sps[:, k * D:(k + 1) * D],
                                            in_=sg[:, k, :],
                                            identity=ident[:])
                    selfT = pool.tile([128, DBLK], F32, tag="selfT")
                    nc.vector.tensor_copy(out=selfT[:], in_=sps[:])

                    # encoder
                    enc_ps = psC.tile([128, DBLK], F32, tag="enc")
                    nc.tensor.matmul(out=enc_ps[:], lhsT=w_self[:],
                                     rhs=selfT[:], start=True, stop=False)
                    nc.tensor.matmul(out=enc_ps[:], lhsT=w_agg[:],
                                     rhs=aggT[:], start=False, stop=True)
                    hT = pool.tile([128, DBLK], F32, tag="hT")
                    nc.scalar.activation(out=hT[:], in_=enc_ps[:], func=relu,
                                         bias=be_sb[:])

                    # transpose h to row-major, store
                    hn = pool.tile([128, DBLK // 128, D], F32, tag="hn")
                    for k in range(DBLK // 128):
                        tr = psT.tile([128, D], F32, tag="tr")
                        nc.tensor.transpose(out=tr[:],
                                            in_=hT[:, k * D:(k + 1) * D],
                                            identity=ident[:])
                        nc.vector.tensor_copy(out=hn[:, k, :], in_=tr[:])
                    nc.sync.dma_start(
                        out=h_out[b * DBLK:(b + 1) * DBLK, :].rearrange(
                            "(j p) f -> p j f", p=128),
                        in_=hn[:])

                    if with_tail:
                        t2_ps = psC.tile([128, DBLK], F32, tag="enc")
                        nc.tensor.matmul(out=t2_ps[:], lhsT=wa2_sb[:],
                                         rhs=hT[:], start=True, stop=True)
                        t2T = pool.tile([128, DBLK], F32, tag="t2T")
                        nc.scalar.activation(out=t2T[:], in_=t2_ps[:],
                                             func=relu, bias=ba2_sb[:])
                        t2n = pool.tile([128, DBLK // 128, D], F32, tag="t2n")
                        for k in range(DBLK // 128):
                            tr = psT.tile([128, D], F32, tag="tr")
                            nc.tensor.transpose(
                                out=tr[:], in_=t2T[:, k * D:(k + 1) * D],
                                identity=ident[:])
                            nc.vector.tensor_copy(out=t2n[:, k, :], in_=tr[:])
                        nc.sync.dma_start(
                            out=t2_out[b * DBLK:(b + 1) * DBLK, :].rearrange(
                                "(j p) f -> p j f", p=128),
                            in_=t2n[:])
    nc.compile()
    return nc


# ===================================================================
# Host orchestration
# ===================================================================
def _shard_pad(arr, ndest_core, pad_to):
    """[N,...] -> [C, pad_to, ...] per-core padded shards."""
    C = NCORES
    out = np.zeros((C, pad_to) + arr.shape[1:], dtype=arr.dtype)
    for c in range(C):
        out[c, :ndest_core] = arr[c * ndest_core:(c + 1) * ndest_core]
    return out


def _run_layer(label, plan, tsrc_full, sself_full, We, be_, Wa2=None,
               ba2=None, with_tail=False):
    nsrc, nself = tsrc_full.shape[0], sself_full.shape[0]
    nc = _build_launch_bc(plan, nsrc, nself, with_tail)
    iota = np.broadcast_to(np.arange(DBLK, dtype=np.float32),
                           (128, DBLK)).copy()
    in_maps = []
    for c in range(NCORES):
        m = {
            "tsrc": tsrc_full, "sself": sself_full,
            "gidx": np.ascontiguousarray(plan.idx16[c]),
            "cw": np.ascontiguousarray(plan.cw[c]),
            "sidx": np.ascontiguousarray(plan.sidx16[c]),
            "sre": np.ascontiguousarray(plan.sre16[c]),
            "iota": iota,
            "wes": We[:D], "wea": We[D:], "be": be_,
        }
        if with_tail:
            m["wa2"] = Wa2
            m["ba2"] = ba2
        in_maps.append(m)
    return _run(nc, in_maps, label)


def kernel(emb, neigh_idx1, neigh_w1, self_idx1, neigh_idx2, neigh_w2,
           self_idx2, Wa1, ba1, We1, be1, Wa2, ba2, We2, be2):
    emb = np.asarray(emb, dtype=np.float32)
    Wa1 = np.asarray(Wa1, dtype=np.float32)
    ba1 = np.asarray(ba1, dtype=np.float32).reshape(D, 1)
    We1 = np.asarray(We1, dtype=np.float32)
    be1 = np.asarray(be1, dtype=np.float32).reshape(D, 1)
    Wa2 = np.asarray(Wa2, dtype=np.float32)
    ba2 = np.asarray(ba2, dtype=np.float32).reshape(D, 1)
    We2 = np.asarray(We2, dtype=np.float32)
    be2 = np.asarray(be2, dtype=np.float32).reshape(D, 1)

    db, dc = N1 // NCORES, N2 // NCORES
    nidx1 = _shard_pad(np.asarray(neigh_idx1, dtype=np.int64), db, DB_DEST)
    sidx1 = _shard_pad(np.asarray(self_idx1, dtype=np.int64), db, DB_DEST)
    nw1 = np.asarray(neigh_w1, dtype=np.float32)
    wn1 = _shard_pad(nw1 / nw1.sum(axis=1, keepdims=True), db, DB_DEST)
    nidx2 = _shard_pad(np.asarray(neigh_idx2, dtype=np.int64), dc, DC_DEST)
    sidx2 = _shard_pad(np.asarray(self_idx2, dtype=np.int64), dc, DC_DEST)
    nw2 = np.asarray(neigh_w2, dtype=np.float32)
    wn2 = _shard_pad(nw2 / nw2.sum(axis=1, keepdims=True), dc, DC_DEST)

    # ---------------- Launch A ----------------
    embT = np.ascontiguousarray(emb.T)
    nc_a = _build_launch_a()
    in_maps = []
    for c in range(NCORES):
        sh = np.zeros((D, PA), dtype=np.float32)
        sh[:, :NA] = embT[:, c * NA:(c + 1) * NA]
        in_maps.append({"embT": sh, "Wa1": Wa1, "ba1": ba1})
    res = _run(nc_a, in_maps, "A")
    t_full = np.concatenate([r["t"][:NA] for r in res], axis=0)

    # ---------------- Launch B ----------------
    plan1 = _build_plan(nidx1, wn1, sidx1, DB_DEST, N0, N0)
    res = _run_layer("B", plan1, t_full, emb, We1, be1, Wa2, ba2,
                     with_tail=True)
    h1_full = np.concatenate([r["h"][:db] for r in res], axis=0)
    t2_full = np.concatenate([r["t2"][:db] for r in res], axis=0)

    # ---------------- Launch C ----------------
    plan2 = _build_plan(nidx2, wn2, sidx2, DC_DEST, N1, N1)
    res = _run_layer("C", plan2, t2_full, h1_full, We2, be2, with_tail=False)
    h2 = np.concatenate([r["h"][:dc] for r in res], axis=0)
    return h2


# revision 10
# speedup vs baseline: 1.0154x; 1.0154x over previous
"""PinSAGE 2-layer GNN on 8 Trainium2 NeuronCores.

Shapes (fixed): N0=400000, N1=100000, N2=20000, F=16, D=128, 8 cores.

Launch A (source-sharded): t = relu(emb @ Wa1 + ba1) over 50000-row shards,
    feature-major matmuls (host supplies emb^T), PE-transpose back to
    row-major so later launches can gather 512B rows.

Launch B (destination-sharded, 12500 dest/core padded to 12800):
    Neighbor aggregation = indirect row gather + weighted sum. The HW gather
    primitive (InstDMAGatherAnt) takes int16 indices, so draws are bucketed
    into 32768-row source windows. Draws are sorted by (256-dest-block,
    window); each (block, window) cell is padded to a multiple of 128 slots
    with dummy row-0/weight-0 draws, sized exactly from the actual inputs
    (max across cores) so one SPMD program fits all 8 cores. Gathers run on
    all 4 SWDGE queues (~4x descriptor throughput). Aggregation runs on the
    PE: per 128-slot tile, stationary = gathered rows, moving = a [128, 256]
    routing matrix built on-device by one DVE tensor_scalar(is_equal, mult)
    from host-sent per-slot (dest-col, weight) pairs; agg^T accumulates
    feature-major in PSUM.
    Self rows use a two-hop gather: window-sorted gather into a <32768-row
    HBM staging table, then a dest-ordered int16 re-gather from it.
    Encoder h1^T = relu(We_s^T self^T + We_a^T agg^T + be), tail
    t2^T = relu(Wa2^T h1^T + ba2), both PE-transposed to row-major.

Launch C: same pipeline for layer 2 (src tables = t2/h1, 2500 dest/core).

Host work is index arithmetic / layout only; all feature data moves and is
computed on device.
"""

import os
import numpy as np

from concourse import bass, bacc, mybir, tile
from concourse.bass_utils import run_bass_kernel_spmd
from concourse.masks import make_identity
import concourse.tile_sem_assignment as _tsa

# Tile assigns the 8 DMASW completion-sem lanes round-robin over Pool DMA
# instructions, but a lane must stay locked to one SWDGE queue. With
# multi-queue gathers, derive the lane from queue_num (2 lanes per queue).
if not getattr(_tsa, "_queue_aware_lanes", False):
    _orig_assign_tick = _tsa.TileClockTick._assign_tick

    def _assign_tick_queue_aware(self, inst):
        qn = getattr(inst, "queue_num", None)
        if (qn is not None and inst.engine == mybir.EngineType.Pool
                and isinstance(inst, _tsa.DMAInst)):
            tog = getattr(self, "_qlane_toggle", None)
            if tog is None:
                tog = self._qlane_toggle = {}
            t = tog.get(qn, 0)
            tog[qn] = t ^ 1
            self.next_sw_dma_idx = 2 * qn + t
        return _orig_assign_tick(self, inst)

    _tsa.TileClockTick._assign_tick = _assign_tick_queue_aware
    _tsa._queue_aware_lanes = True

F32 = mybir.dt.float32
BF16 = mybir.dt.bfloat16
I16 = mybir.dt.int16

N0, N1, N2, F, D = 400000, 100000, 20000, 16, 128
NCORES = 8
WSZ = 32768          # int16-addressable source window
DBLK = 256           # dest block = routing matmul N
MAXCALL = 1024       # max indices per dma_gather call (descriptor ring)

PA = 50176           # launch A padded shard rows (392*128)
NA = N0 // NCORES
DB_DEST = 12800      # launch B padded dest/core
DC_DEST = 2560       # launch C padded dest/core

LAST_EXEC_TIMES = {}
LAST_PROFILES = {}


def cdiv(a, b):
    return -(-a // b)


def _run(nc, in_maps, label):
    want_trace = bool(os.environ.get("BASS_TRACE"))
    res = run_bass_kernel_spmd(nc, in_maps, core_ids=list(range(NCORES)),
                               trace=want_trace)
    if want_trace:
        LAST_EXEC_TIMES[label] = res.exec_time_ns
        if res.instructions_and_trace is not None:
            LAST_PROFILES[label] = (res.instructions_and_trace[1],
                                    res.profile_json)
    return res.results


def idx16_cols(idxs):
    """Flat int16 index list (len % 16 == 0) -> [128, len/16] wrapped layout:
    index i read from [i % 16, i // 16]; replicated to all 8 Q7 groups."""
    a = np.asarray(idxs, dtype=np.int16)
    t16 = a.reshape(-1, 16).T  # [16, cols]
    return np.tile(t16, (8, 1))  # [128, cols]


# ===================================================================
# Launch A
# ===================================================================
def _build_launch_a(pa=PA):
    nc = bacc.Bacc("TRN2", target_bir_lowering=False, debug=False)
    embT = nc.dram_tensor("embT", [D, pa], F32, kind="ExternalInput").ap()
    Wa1 = nc.dram_tensor("Wa1", [D, D], F32, kind="ExternalInput").ap()
    ba1 = nc.dram_tensor("ba1", [D, 1], F32, kind="ExternalInput").ap()
    t_out = nc.dram_tensor("t", [pa, D], F32, kind="ExternalOutput").ap()

    CH = 8            # 8 node-tiles (1024 nodes) per DMA chunk
    nchunks = pa // (D * CH)

    with tile.TileContext(nc) as tc:
        with tc.tile_pool(name="const", bufs=1) as cpool, \
             tc.tile_pool(name="sb", bufs=3) as pool, \
             tc.tile_pool(name="ps", bufs=4, space="PSUM") as psum:
            ident = cpool.tile([D, D], F32)
            make_identity(nc, ident[:])
            wa = cpool.tile([D, D], F32)
            nc.sync.dma_start(out=wa[:], in_=Wa1[:])
            ba = cpool.tile([D, 1], F32)
            nc.sync.dma_start(out=ba[:], in_=ba1[:])

            for i in range(nchunks):
                x = pool.tile([D, CH * D], F32, tag="x")
                nc.sync.dma_start(
                    out=x[:], in_=embT[:, i * CH * D:(i + 1) * CH * D])
                tn = pool.tile([D, CH, D], F32, tag="tn")
                for j in range(0, CH, 4):
                    ps1 = psum.tile([D, 4 * D], F32, tag="mm")
                    nc.tensor.matmul(out=ps1[:],
                                     lhsT=wa[:],
                                     rhs=x[:, j * D:(j + 4) * D],
                                     start=True, stop=True)
                    tT = pool.tile([D, 4 * D], F32, tag="tT")
                    nc.scalar.activation(
                        out=tT[:], in_=ps1[:],
                        func=mybir.ActivationFunctionType.Relu, bias=ba[:])
                    for k in range(4):
                        ps2 = psum.tile([D, D], F32, tag="tr")
                        nc.tensor.transpose(out=ps2[:],
                                            in_=tT[:, k * D:(k + 1) * D],
                                            identity=ident[:])
                        nc.vector.tensor_copy(out=tn[:, j + k, :], in_=ps2[:])
                nc.sync.dma_start(
                    out=t_out[i * CH * D:(i + 1) * CH * D, :].rearrange(
                        "(j p) f -> p j f", p=D),
                    in_=tn[:])
    nc.compile()
    return nc


# ===================================================================
# Plan for launches B/C: window-bucketed gather layout
# ===================================================================
class Plan:
    pass


def _build_plan(nidx_all, nw_all, sidx_all, ndest, nsrc, nself):
    """nidx_all [C, ndest, F] int64, nw_all [C, ndest, F] f32 (normalized,
    pads zero), sidx_all [C, ndest] int64. Returns a Plan with a static
    structure (shared by all cores, sized from per-cell maxima) plus
    per-core index/weight tensors."""
    C = NCORES
    W = cdiv(nsrc, WSZ)
    nblk = ndest // DBLK
    npair = cdiv(nblk, 2)
    draws = ndest * F

    dest = np.repeat(np.arange(ndest), F)
    blk_of_draw = dest >> 8

    # ---- per-core cell sort (cell = (block, window)) ----
    cell_sorted = []   # per core: (order, cellid)
    counts = np.zeros((C, nblk, W), dtype=np.int64)
    for c in range(C):
        src = nidx_all[c].reshape(-1)
        w = src >> 15
        cellid = blk_of_draw * W + w
        order = np.argsort(cellid, kind="stable")  # keeps (dest, f) order
        cell_sorted.append((order, cellid))
        cnt = np.bincount(cellid, minlength=nblk * W).reshape(nblk, W)
        counts[c] = cnt

    S = (cdiv(counts.max(axis=0), 128) * 128).astype(np.int64)  # [nblk, W]

    # ---- static layout: per pair, per window: cells of its (1-2) blocks ----
    plan = Plan()
    plan.W, plan.nblk, plan.npair, plan.ndest, plan.nsrc = W, nblk, npair, ndest, nsrc
    plan.pairs = []
    cw_tiles = []      # traversal order of (pair, w, b, tile) -> cw column
    idx_col = 0        # column offset into the global idx16 tensor
    for p in range(npair):
        blocks = [b for b in (2 * p, 2 * p + 1) if b < nblk]
        info = {"blocks": {b: [] for b in blocks}, "calls": [],
                "idx_col0": idx_col, "nslots": 0}
        slot_off = 0
        for w in range(W):
            ncall = int(sum(S[b, w] for b in blocks))
            if ncall == 0:
                continue
            # split at MAXCALL boundaries (cell sizes are 128-multiples)
            seg = []
            acc = 0
            for b in blocks:
                sbw = int(S[b, w])
                if sbw:
                    info["blocks"][b].append((w, slot_off + acc, sbw))
                    acc += sbw
            start = 0
            while start < ncall:
                n = min(MAXCALL, ncall - start)
                info["calls"].append(
                    dict(w=w, n=n, dst_slot=slot_off + start,
                         idx_col=idx_col + start // 16))
                start += n
            slot_off += ncall
            idx_col += ncall // 16
        info["nslots"] = slot_off
        # tile list per block (slot offsets of each 128-slot tile)
        for b in blocks:
            tl = []
            for (w, off, sbw) in info["blocks"][b]:
                for t0 in range(off, off + sbw, 128):
                    tl.append((t0, len(cw_tiles)))
                    cw_tiles.append(None)  # placeholder; filled per core
            info["blocks"][b] = tl
        plan.pairs.append(info)
    plan.idx_cols_total = idx_col
    plan.ntiles_total = len(cw_tiles)
    plan.maxpair_slots = max(i["nslots"] for i in plan.pairs)
    plan.maxpair_idxcols = max(
        (sum(cc["n"] for cc in i["calls"])) // 16 for i in plan.pairs)
    plan.maxpair_tiles = max(
        sum(len(tl) for tl in i["blocks"].values()) for i in plan.pairs)

    # ---- per-core idx16 + cw tensors ----
    plan.idx16 = np.zeros((C, 128, idx_col), dtype=np.int16)
    plan.cw = np.zeros((C, 128, 2 * plan.ntiles_total), dtype=np.float32)
    for c in range(C):
        order, cellid = cell_sorted[c]
        src = nidx_all[c].reshape(-1)
        wt = nw_all[c].reshape(-1)
        dst_col = dest & (DBLK - 1)
        # per cell: sorted draw lists
        cell_start = np.zeros(nblk * W + 1, dtype=np.int64)
        np.cumsum(np.bincount(cellid, minlength=nblk * W), out=cell_start[1:])
        for p, info in enumerate(plan.pairs):
            # idx stream: per window, blocks in order, cell draws + pads
            stream = []
            for w in range(W):
                for b in (2 * p, 2 * p + 1):
                    if b >= nblk or S[b, w] == 0:
                        continue
                    cid = b * W + w
                    dr = order[cell_start[cid]:cell_start[cid + 1]]
                    loc = src[dr] - (w << 15)
                    pad = int(S[b, w]) - len(dr)
                    stream.append(loc)
                    if pad:
                        stream.append(np.zeros(pad, dtype=np.int64))
            if stream:
                flat = np.concatenate(stream)
                assert len(flat) == info["nslots"]
                cols = idx16_cols(flat)
                c0 = info["idx_col0"]
                plan.idx16[c, :, c0:c0 + len(flat) // 16] = cols
        # fill cw via a slot->draw map per pair
        for p, info in enumerate(plan.pairs):
            nslots = info["nslots"]
            slot_dstcol = np.zeros(nslots, dtype=np.float32)
            slot_w = np.zeros(nslots, dtype=np.float32)
            pos = 0
            for w in range(W):
                for b in (2 * p, 2 * p + 1):
                    if b >= nblk or S[b, w] == 0:
                        continue
                    cid = b * W + w
                    dr = order[cell_start[cid]:cell_start[cid + 1]]
                    n = len(dr)
                    slot_dstcol[pos:pos + n] = dst_col[dr]
                    slot_w[pos:pos + n] = wt[dr]
                    pos += int(S[b, w])
            for b, tl in info["blocks"].items():
                for (t0, cwcol) in tl:
                    plan.cw[c, :, 2 * cwcol] = slot_dstcol[t0:t0 + 128]
                    plan.cw[c, :, 2 * cwcol + 1] = slot_w[t0:t0 + 128]

    # ---- self rows: two-hop ----
    WS = cdiv(nself, WSZ)
    plan.WS = WS
    scnt = np.zeros((C, WS), dtype=np.int64)
    sorders = []
    for c in range(C):
        sw = sidx_all[c] >> 15
        so = np.argsort(sw, kind="stable")
        sorders.append(so)
        scnt[c] = np.bincount(sw, minlength=WS)
    SW = (cdiv(scnt.max(axis=0), 128) * 128).astype(np.int64)  # [WS]
    plan.SW = SW
    plan.stage_rows = int(SW.sum())
    soff = np.concatenate([[0], np.cumsum(SW)])
    plan.self_calls = []
    for w in range(WS):
        if SW[w] == 0:
            continue
        start = 0
        while start < SW[w]:
            n = int(min(MAXCALL, SW[w] - start))
            plan.self_calls.append(
                dict(w=w, n=n, dst_row=int(soff[w] + start),
                     idx_col=int((soff[w] + start) // 16)))
            start += n
    plan.sidx16 = np.zeros((C, 128, plan.stage_rows // 16), dtype=np.int16)
    plan.sre16 = np.zeros((C, 128, ndest // 16), dtype=np.int16)
    for c in range(C):
        so = sorders[c]
        sw = (sidx_all[c] >> 15)[so]
        vals = sidx_all[c][so] - (sw << 15)
        stream = []
        p_stage = np.zeros(ndest, dtype=np.int64)
        for w in range(WS):
            n = int(scnt[c, w])
            seg = vals[np.searchsorted(sw, w):np.searchsorted(sw, w + 1)]
            dests = so[np.searchsorted(sw, w):np.searchsorted(sw, w + 1)]
            p_stage[dests] = soff[w] + np.arange(n)
            stream.append(seg)
            stream.append(np.zeros(int(SW[w]) - n, dtype=np.int64))
        flat = np.concatenate(stream) if stream else np.zeros(0)
        plan.sidx16[c] = idx16_cols(flat)
        plan.sre16[c] = idx16_cols(p_stage)
    return plan


# ===================================================================
# Launch B/C kernel builder
# ===================================================================
def _build_launch_bc(plan, nsrc, nself, with_tail):
    ndest, W, nblk = plan.ndest, plan.W, plan.nblk

    nc = bacc.Bacc("TRN2", target_bir_lowering=False, debug=False,
                   num_swdge_queues=4)
    tsrc = nc.dram_tensor("tsrc", [nsrc, D], F32, kind="ExternalInput").ap()
    sself = nc.dram_tensor("sself", [nself, D], F32, kind="ExternalInput").ap()
    gidx = nc.dram_tensor("gidx", [128, plan.idx_cols_total], I16,
                          kind="ExternalInput").ap()
    cw = nc.dram_tensor("cw", [128, 2 * plan.ntiles_total], F32,
                        kind="ExternalInput").ap()
    sidx = nc.dram_tensor("sidx", [128, plan.stage_rows // 16], I16,
                          kind="ExternalInput").ap()
    sre = nc.dram_tensor("sre", [128, ndest // 16], I16,
                         kind="ExternalInput").ap()
    iota = nc.dram_tensor("iota", [128, DBLK], F32, kind="ExternalInput").ap()
    wes = nc.dram_tensor("wes", [D, D], F32, kind="ExternalInput").ap()
    wea = nc.dram_tensor("wea", [D, D], F32, kind="ExternalInput").ap()
    be = nc.dram_tensor("be", [D, 1], F32, kind="ExternalInput").ap()
    h_out = nc.dram_tensor("h", [ndest, D], F32, kind="ExternalOutput").ap()
    if with_tail:
        wa2 = nc.dram_tensor("wa2", [D, D], F32, kind="ExternalInput").ap()
        ba2 = nc.dram_tensor("ba2", [D, 1], F32, kind="ExternalInput").ap()
        t2_out = nc.dram_tensor("t2", [ndest, D], F32,
                                kind="ExternalOutput").ap()
    stage = nc.dram_tensor("stage", [plan.stage_rows, D], F32).ap()

    qload = [0, 0, 0, 0]

    def next_q(n=MAXCALL):
        q = qload.index(min(qload))
        qload[q] += n
        return q

    def win_view(tensor, rows, w):
        lo = w * WSZ
        hi = min(rows, lo + WSZ)
        return tensor[lo:hi, :]

    relu = mybir.ActivationFunctionType.Relu

    with tile.TileContext(nc) as tc:
        with tc.tile_pool(name="const", bufs=1) as cpool, \
             tc.tile_pool(name="gt", bufs=2) as gpool, \
             tc.tile_pool(name="sb", bufs=2) as pool, \
             tc.tile_pool(name="rt", bufs=4) as rpool, \
             tc.tile_pool(name="psA", bufs=2, space="PSUM") as psA, \
             tc.tile_pool(name="psB", bufs=2, space="PSUM") as psB, \
             tc.tile_pool(name="psC", bufs=2, space="PSUM") as psC, \
             tc.tile_pool(name="psT", bufs=2, space="PSUM") as psT:
            ident = cpool.tile([D, D], F32)
            make_identity(nc, ident[:])
            w_self = cpool.tile([D, D], F32)
            nc.sync.dma_start(out=w_self[:], in_=wes[:])
            w_agg = cpool.tile([D, D], F32)
            nc.sync.dma_start(out=w_agg[:], in_=wea[:])
            be_sb = cpool.tile([D, 1], F32)
            nc.sync.dma_start(out=be_sb[:], in_=be[:])
            iota_sb = cpool.tile([128, DBLK], F32)
            nc.sync.dma_start(out=iota_sb[:], in_=iota[:])
            if with_tail:
                wa2_sb = cpool.tile([D, D], F32)
                nc.sync.dma_start(out=wa2_sb[:], in_=wa2[:])
                ba2_sb = cpool.tile([D, 1], F32)
                nc.sync.dma_start(out=ba2_sb[:], in_=ba2[:])
            sre_sb = cpool.tile([128, ndest // 16], I16)
            nc.sync.dma_start(out=sre_sb[:], in_=sre[:])

            # ---- phase S1: self rows -> window-sorted staging table ----
            sidx_sb = cpool.tile([128, plan.stage_rows // 16], I16)
            nc.sync.dma_start(out=sidx_sb[:], in_=sidx[:])
            for call in plan.self_calls:
                n = call["n"]
                st = pool.tile([128, MAXCALL // 128, D], F32, tag="s1")
                nc.gpsimd.dma_gather(
                    out_ap=st[:, :n // 128, :],
                    in_ap=win_view(sself, nself, call["w"]),
                    idxs_ap=sidx_sb[:, call["idx_col"]:call["idx_col"] + n // 16],
                    num_idxs=n, num_idxs_reg=n, elem_size=D,
                    queue_num=next_q())
                nc.sync.dma_start(
                    out=stage[call["dst_row"]:call["dst_row"] + n, :].rearrange(
                        "(j p) f -> p j f", p=128),
                    in_=st[:, :n // 128, :])

            # ---- main loop over block pairs ----
            for p, info in enumerate(plan.pairs):
                nslots = info["nslots"]
                if nslots == 0:
                    continue
                icols = nslots // 16
                gi = pool.tile([128, plan.maxpair_idxcols], I16, tag="gi")
                c0 = info["idx_col0"]
                nc.sync.dma_start(out=gi[:, :icols],
                                  in_=gidx[:, c0:c0 + icols])
                gt = gpool.tile([128, plan.maxpair_slots // 128, D], F32,
                                tag="gt")
                for call in info["calls"]:
                    n, ds = call["n"], call["dst_slot"]
                    nc.gpsimd.dma_gather(
                        out_ap=gt[:, ds // 128:(ds + n) // 128, :],
                        in_ap=win_view(tsrc, nsrc, call["w"]),
                        idxs_ap=gi[:, call["idx_col"] - c0:
                                   call["idx_col"] - c0 + n // 16],
                        num_idxs=n, num_idxs_reg=n, elem_size=D,
                        queue_num=next_q())
                # cw chunk for this pair
                tiles_all = [t for b in info["blocks"]
                             for t in info["blocks"][b]]
                cw0 = min(t[1] for t in tiles_all)
                cwn = len(tiles_all)
                cwt = pool.tile([128, 2 * plan.maxpair_tiles], F32, tag="cw")
                nc.sync.dma_start(out=cwt[:, :2 * cwn],
                                  in_=cw[:, 2 * cw0:2 * (cw0 + cwn)])

                for b, tl in info["blocks"].items():
                    if not tl:
                        continue
                    # aggregation: w-prescale+cast on ACT, 0/1 mask on DVE
                    agg_ps = psA.tile([128, DBLK], F32, tag="agg")
                    for j, (t0, cwcol) in enumerate(tl):
                        lc = 2 * (cwcol - cw0)
                        sc = rpool.tile([128, D], BF16, tag="sc")
                        nc.scalar.activation(
                            out=sc[:], in_=gt[:, t0 // 128, :],
                            func=mybir.ActivationFunctionType.Copy,
                            scale=cwt[:, lc + 1:lc + 2])
                        route = rpool.tile([128, DBLK], BF16, tag="route")
                        nc.vector.tensor_scalar(
                            out=route[:], in0=iota_sb[:],
                            scalar1=cwt[:, lc:lc + 1], scalar2=None,
                            op0=mybir.AluOpType.is_equal)
                        nc.tensor.matmul(
                            out=agg_ps[:], lhsT=sc[:],
                            rhs=route[:], start=(j == 0),
                            stop=(j == len(tl) - 1))
                    aggT = pool.tile([128, DBLK], F32, tag="aggT")
                    nc.vector.tensor_copy(out=aggT[:], in_=agg_ps[:])

                    # self re-gather (dest order) + transpose
                    sg = pool.tile([128, DBLK // 128, D], F32, tag="sg")
                    nc.gpsimd.dma_gather(
                        out_ap=sg[:], in_ap=stage[:],
                        idxs_ap=sre_sb[:, b * DBLK // 16:(b + 1) * DBLK // 16],
                        num_idxs=DBLK, num_idxs_reg=DBLK, elem_size=D,
                        queue_num=next_q())
                    sps = psB.tile([128, DBLK], F32, tag="selfps")
                    for k in range(DBLK // 128):
                        nc.tensor.transpose(out=sps[:, k * D:(k + 1) * D],
                                            in_=sg[:, k, :],
                                            identity=ident[:])
                    selfT = pool.tile([128, DBLK], F32, tag="selfT")
                    nc.vector.tensor_copy(out=selfT[:], in_=sps[:])

                    # encoder
                    enc_ps = psC.tile([128, DBLK], F32, tag="enc")
                    nc.tensor.matmul(out=enc_ps[:], lhsT=w_self[:],
                                     rhs=selfT[:], start=True, stop=False)
                    nc.tensor.matmul(out=enc_ps[:], lhsT=w_agg[:],
                                     rhs=aggT[:], start=False, stop=True)
                    hT = pool.tile([128, DBLK], F32, tag="hT")
                    nc.scalar.activation(out=hT[:], in_=enc_ps[:], func=relu,
                                         bias=be_sb[:])

                    # transpose h to row-major, store
                    hn = pool.tile([128, DBLK // 128, D], F32, tag="hn")
                    for k in range(DBLK // 128):
                        tr = psT.tile([128, D], F32, tag="tr")
                        nc.tensor.transpose(out=tr[:],
                                            in_=hT[:, k * D:(k + 1) * D],
                                            identity=ident[:])
                        nc.vector.tensor_copy(out=hn[:, k, :], in_=tr[:])
                    nc.sync.dma_start(
                        out=h_out[b * DBLK:(b + 1) * DBLK, :].rearrange(
                            "(j p) f -> p j f", p=128),
                        in_=hn[:])

                    if with_tail:
                        t2_ps = psC.tile([128, DBLK], F32, tag="enc")
                        nc.tensor.matmul(out=t2_ps[:], lhsT=wa2_sb[:],
                                         rhs=hT[:], start=True, stop=True)
                        t2T = pool.tile([128, DBLK], F32, tag="t2T")
                        nc.scalar.activation(out=t2T[:], in_=t2_ps[:],
                                             func=relu, bias=ba2_sb[:])
                        t2n = pool.tile([128, DBLK // 128, D], F32, tag="t2n")
                        for k in range(DBLK // 128):
                            tr = psT.tile([128, D], F32, tag="tr")
                            nc.tensor.transpose(
                                out=tr[:], in_=t2T[:, k * D:(k + 1) * D],
                                identity=ident[:])
                            nc.vector.tensor_copy(out=t2n[:, k, :], in_=tr[:])
                        nc.sync.dma_start(
                            out=t2_out[b * DBLK:(b + 1) * DBLK, :].rearrange(
                                "(j p) f -> p j f", p=128),
                            in_=t2n[:])
    nc.compile()
    return nc


# ===================================================================
# Host orchestration
# ===================================================================
def _shard_pad(arr, ndest_core, pad_to):
    """[N,...] -> [C, pad_to, ...] per-core padded shards."""
    C = NCORES
    out = np.zeros((C, pad_to) + arr.shape[1:], dtype=arr.dtype)
    for c in range(C):
        out[c, :ndest_core] = arr[c * ndest_core:(c + 1) * ndest_core]
    return out


def _run_layer(label, plan, tsrc_full, sself_full, We, be_, Wa2=None,
               ba2=None, with_tail=False):
    nsrc, nself = tsrc_full.shape[0], sself_full.shape[0]
    nc = _build_launch_bc(plan, nsrc, nself, with_tail)
    iota = np.broadcast_to(np.arange(DBLK, dtype=np.float32),
                           (128, DBLK)).copy()
    in_maps = []
    for c in range(NCORES):
        m = {
            "tsrc": tsrc_full, "sself": sself_full,
            "gidx": np.ascontiguousarray(plan.idx16[c]),
            "cw": np.ascontiguousarray(plan.cw[c]),
            "sidx": np.ascontiguousarray(plan.sidx16[c]),
            "sre": np.ascontiguousarray(plan.sre16[c]),
            "iota": iota,
            "wes": We[:D], "wea": We[D:], "be": be_,
        }
        if with_tail:
            m["wa2"] = Wa2
            m["ba2"] = ba2
        in_maps.append(m)
    return _run(nc, in_maps, label)


def kernel(emb, neigh_idx1, neigh_w1, self_idx1, neigh_idx2, neigh_w2,
           self_idx2, Wa1, ba1, We1, be1, Wa2, ba2, We2, be2):
    emb = np.asarray(emb, dtype=np.float32)
    Wa1 = np.asarray(Wa1, dtype=np.float32)
    ba1 = np.asarray(ba1, dtype=np.float32).reshape(D, 1)
    We1 = np.asarray(We1, dtype=np.float32)
    be1 = np.asarray(be1, dtype=np.float32).reshape(D, 1)
    Wa2 = np.asarray(Wa2, dtype=np.float32)
    ba2 = np.asarray(ba2, dtype=np.float32).reshape(D, 1)
    We2 = np.asarray(We2, dtype=np.float32)
    be2 = np.asarray(be2, dtype=np.float32).reshape(D, 1)

    db, dc = N1 // NCORES, N2 // NCORES
    nidx1 = _shard_pad(np.asarray(neigh_idx1, dtype=np.int64), db, DB_DEST)
    sidx1 = _shard_pad(np.asarray(self_idx1, dtype=np.int64), db, DB_DEST)
    nw1 = np.asarray(neigh_w1, dtype=np.float32)
    wn1 = _shard_pad(nw1 / nw1.sum(axis=1, keepdims=True), db, DB_DEST)
    nidx2 = _shard_pad(np.asarray(neigh_idx2, dtype=np.int64), dc, DC_DEST)
    sidx2 = _shard_pad(np.asarray(self_idx2, dtype=np.int64), dc, DC_DEST)
    nw2 = np.asarray(neigh_w2, dtype=np.float32)
    wn2 = _shard_pad(nw2 / nw2.sum(axis=1, keepdims=True), dc, DC_DEST)

    # ---------------- Launch A ----------------
    embT = np.ascontiguousarray(emb.T)
    nc_a = _build_launch_a()
    in_maps = []
    for c in range(NCORES):
        sh = np.zeros((D, PA), dtype=np.float32)
        sh[:, :NA] = embT[:, c * NA:(c + 1) * NA]
        in_maps.append({"embT": sh, "Wa1": Wa1, "ba1": ba1})
    res = _run(nc_a, in_maps, "A")
    t_full = np.concatenate([r["t"][:NA] for r in res], axis=0)

    # ---------------- Launch B ----------------
    plan1 = _build_plan(nidx1, wn1, sidx1, DB_DEST, N0, N0)
    res = _run_layer("B", plan1, t_full, emb, We1, be1, Wa2, ba2,
                     with_tail=True)
    h1_full = np.concatenate([r["h"][:db] for r in res], axis=0)
    t2_full = np.concatenate([r["t2"][:db] for r in res], axis=0)

    # ---------------- Launch C ----------------
    plan2 = _build_plan(nidx2, wn2, sidx2, DC_DEST, N1, N1)
    res = _run_layer("C", plan2, t2_full, h1_full, We2, be2, with_tail=False)
    h2 = np.concatenate([r["h"][:dc] for r in res], axis=0)
    return h2
